# revision 27
# baseline (speedup 1.0000x reference)
"""Trainium2 Bass kernel for a causal self-attention block (v2).

Reference computation (fp32):
    qkv = x @ W_qkv + b_qkv ; q,k,v = split(qkv)
    scores = (q @ k.T + mask) / sqrt(hd)
    wts = exp(scores) / (sum(exp(scores)) + 1e-9)
    y = (wts @ v) @ W_out + b_out
    out = LayerNorm(x + y) * gamma + beta

Sharding: 8 cores = 4 batches x 2 query-parities.  Core (b, sh) handles
queries sh::2 of batch b (stride-2 interleave).  With a causal mask this
makes every 256-query chunk c need exactly k-tiles 0..4c+3, identically
on both parities, so the block schedule is core-uniform with no padding,
and only 4 distinct diagonal-band mask tiles exist (block offset d=0..3,
independent of the chunk).

All matmuls run in bf16.  Everything stays in SBUF: K^T, V (token-major
with a ones column so the softmax denominator falls out of the attention
matmul), Q^T (whose pool slots are reused for the attention output), and
z for the LayerNorm re-read.  x is streamed per 512-token quarter and
K/V projections are fused over one pass.  Scores for a head pair run as
two concurrent row-tiled matmuls (contraction 64 on partitions 0-63 /
64-127).  The additive mask is seeded into the scores PSUM bank by an
identity matmul (start=True) that the score matmul then accumulates onto
(has_written semantics), so exp feeds the attention matmul directly.
Partition broadcasts (softmax 1/den, LayerNorm mean/rstd) run on the
otherwise idle GpSimd engine instead of PE ones-matmuls, which keeps the
PE FIFO off the DVE reduction chains.  LayerNorm feature-axis sums use
ones-vector matmuls; the residual reuses the resident bf16 x tiles; the
output is written bf16 and widened to fp32 on the host.
"""

import numpy as np
import ml_dtypes

import concourse.bass as bass
import concourse.tile as tile
from concourse import bacc, mybir
from concourse.bass_utils import run_bass_kernel_spmd

# Problem dims (hardcoded per harness contract)
B, S, D, H = 4, 2048, 1024, 16
HD = D // H                      # 64 head dim
HE = HD + 1                      # + ones column
N_CORES = 8
SQ = S // 2                      # queries per core
QC = 256                         # query chunk (mask granularity)
NCH = SQ // QC                   # 4 chunks per core
KT = 128                         # k rows per tile
NKT = S // KT                    # 16 k tiles
NF = D // 128                    # 8 feature tiles
NDC = D // 128                   # 8 contraction tiles
SM_EPS = 1.0e-9
LN_EPS = 1.0e-5

F32 = mybir.dt.float32
BF16 = mybir.dt.bfloat16
ALU = mybir.AluOpType
ACTF = mybir.ActivationFunctionType
BF = ml_dtypes.bfloat16


def _build_program(n_iters=1, timing_mode=False, phases=("p1", "p2", "p34")):
    nc = bacc.Bacc("TRN2", target_bir_lowering=False, debug=False,
                   num_devices=N_CORES)

    big = "Internal" if timing_mode else "ExternalInput"
    xT_b = nc.dram_tensor("xT_b", [D, S], BF16, kind=big).ap()
    xqT_b = nc.dram_tensor("xqT_b", [D, SQ], BF16, kind=big).ap()
    wq_b = nc.dram_tensor("wq_b", [D, D], BF16, kind=big).ap()
    wk_b = nc.dram_tensor("wk_b", [D, D], BF16, kind=big).ap()
    wv_b = nc.dram_tensor("wv_b", [D, D], BF16, kind=big).ap()
    wo_b = nc.dram_tensor("wo_b", [D, D], BF16, kind=big).ap()
    cstf = nc.dram_tensor("cstf", [128, 5 * NF], F32,
                          kind="ExternalInput").ap()
    cstb = nc.dram_tensor("cstb", [128, 2304], BF16,
                          kind="ExternalInput").ap()
    yT = nc.dram_tensor("yT", [D, SQ], BF16, kind="ExternalOutput").ap()

    def emit(tc, P):
        ctx_lp = nc.allow_low_precision(reason="bf16 matmul chain")
        ctx_lp.__enter__()
        U = P["u"]
        do_p1 = "p1" in phases
        do_p2 = "p2" in phases
        do_p3 = "p34" in phases or "p3" in phases
        if "p34" in phases or "p4" in phases:
            p4_level = 4
        elif "p4nd" in phases:
            p4_level = 3
        elif "p4sb" in phases:
            p4_level = 2
        elif "p4s" in phases:
            p4_level = 1
        else:
            p4_level = 0
        do_p4 = p4_level > 0

        # ---- constants (packed: 3 DMAs total) -------------------------
        cstf_sb = U.tile([128, 5 * NF], F32, name="cstf", tag="cstf", bufs=1)
        nc.sync.dma_start(cstf_sb[:], cstf[:])
        cstb_sb = U.tile([128, 2304], BF16, name="cstb", tag="cstb", bufs=1)
        nc.sync.dma_start(cstb_sb[:], cstb[:])
        bq_sb = [cstf_sb[:, 0 * NF + f:0 * NF + f + 1] for f in range(NF)]
        bk_sb = [cstf_sb[:, 1 * NF + f:1 * NF + f + 1] for f in range(NF)]
        bo_sb = [cstf_sb[:, 2 * NF + f:2 * NF + f + 1] for f in range(NF)]
        ga_sb = [cstf_sb[:, 3 * NF + f:3 * NF + f + 1] for f in range(NF)]
        be_sb = [cstf_sb[:, 4 * NF + f:4 * NF + f + 1] for f in range(NF)]
        bv_sb = cstb_sb[:, 0:D]
        mask_sb = [cstb_sb[:, D + d * QC:D + (d + 1) * QC] for d in range(4)]
        ones_sb = cstb_sb[:, 2048:2049]
        id_sb = cstb_sb[:, 2176:2304]
        eps_sb = U.tile([1, 1], F32, name="eps", tag="eps", bufs=1)
        nc.vector.memset(eps_sb[:], LN_EPS)

        # ---- persistent SBUF state ------------------------------------
        # Q^T tiles first occupy the qa slots; the attention output aT
        # reuses freed slots (same tag).
        kt_sb = [U.tile([128, S], BF16, name="kt", tag="kt", bufs=NF)
                 for _ in range(NF)]
        v_sb = [U.tile([128, H, HE], BF16, name="vv", tag="vv", bufs=NKT)
                for _ in range(NKT)]
        z_sb = [U.tile([128, SQ], BF16, name="zz", tag="zz", bufs=NF)
                for _ in range(NF)]

        qa_bufs = NF + 1
        q_sb = [U.tile([128, SQ], BF16, name="qa", tag="qa", bufs=qa_bufs)
                for _ in range(NF)]

        # ---- P1a: Q^T projection --------------------------------------
        if do_p1:
            wq_sb = [U.tile([128, D], BF16, name="ww", tag="ww", bufs=16)
                     for _ in range(NDC)]
            xq_sb = [U.tile([128, SQ], BF16, name="xq", tag="xq", bufs=NDC)
                     for _ in range(NDC)]
            for dc in range(NDC):
                s = dc * 128
                nc.sync.dma_start(wq_sb[dc][:], wq_b[s:s + 128, :])
                nc.sync.dma_start(xq_sb[dc][:], xqT_b[s:s + 128, :])
            for f in range(NF):
                ps = P["psS"].tile([128, 1024], F32, name="psS", tag="psS")
                for c in range(2):
                    for dc in range(NDC):
                        nc.tensor.matmul(
                            ps[:, c * 512:(c + 1) * 512],
                            wq_sb[dc][:, f * 128:(f + 1) * 128],
                            xq_sb[dc][:, c * 512:(c + 1) * 512],
                            start=(dc == 0), stop=(dc == NDC - 1))
                nc.vector.tensor_scalar_add(q_sb[f][:], ps[:], bq_sb[f])

        # ---- P1b: fused K^T + V projection, streamed x quarters -------
        if do_p1:
            wk_sb = [U.tile([128, D], BF16, name="ww", tag="ww", bufs=16)
                     for _ in range(NDC)]
            wv_sb = [U.tile([128, D], BF16, name="wv", tag="zz", bufs=NF)
                     for _ in range(NDC)]
            for dc in range(NDC):
                s = dc * 128
                nc.sync.dma_start(wk_sb[dc][:], wk_b[s:s + 128, :])
                nc.sync.dma_start(wv_sb[dc][:], wv_b[s:s + 128, :])
            for qt in range(4):
                ts = qt * 512
                xt_sb = [U.tile([128, 512], BF16, name="xt", tag="xt",
                                bufs=10) for _ in range(NDC)]
                for dc in range(NDC):
                    nc.sync.dma_start(
                        xt_sb[dc][:], xT_b[dc * 128:(dc + 1) * 128,
                                           ts:ts + 512])
                # K^T: two f-tiles per PSUM tile
                for fp in range(NF // 2):
                    ps = P["psS"].tile([128, 1024], F32, name="psS",
                                       tag="psS")
                    for half in range(2):
                        f = 2 * fp + half
                        for dc in range(NDC):
                            nc.tensor.matmul(
                                ps[:, half * 512:(half + 1) * 512],
                                wk_sb[dc][:, f * 128:(f + 1) * 128],
                                xt_sb[dc][:],
                                start=(dc == 0), stop=(dc == NDC - 1))
                    for half in range(2):
                        f = 2 * fp + half
                        nc.vector.tensor_scalar_add(
                            kt_sb[f][:, ts:ts + 512],
                            ps[:, half * 512:(half + 1) * 512], bk_sb[f])
                # V token-major: per 128-token slice
                for sl in range(4):
                    vt = v_sb[qt * 4 + sl]
                    ps = P["psS"].tile([128, 1024], F32, name="psS",
                                       tag="psS")
                    for fc in range(2):
                        for dc in range(NDC):
                            nc.tensor.matmul(
                                ps[:, fc * 512:(fc + 1) * 512],
                                xt_sb[dc][:, sl * 128:(sl + 1) * 128],
                                wv_sb[dc][:, fc * 512:(fc + 1) * 512],
                                start=(dc == 0), stop=(dc == NDC - 1))
                    for fc in range(2):
                        nc.vector.tensor_add(
                            vt[:, fc * 8:(fc + 1) * 8, 0:HD],
                            ps[:, fc * 512:(fc + 1) * 512]
                                .rearrange("p (h e) -> p h e", e=HD),
                            bv_sb[:, fc * 512:(fc + 1) * 512]
                                .rearrange("p (h e) -> p h e", e=HD))
                    nc.vector.memset(vt[:, :, HD:HE], 1.0)

        # prefetch W_out into slots freed by Wq/Wk at P1 end
        wo_sb = []
        if do_p3:
            wo_sb = [U.tile([128, D], BF16, name="ww", tag="ww", bufs=16)
                     for _ in range(NDC)]
            for dc in range(NDC):
                nc.sync.dma_start(wo_sb[dc][:],
                                  wo_b[dc * 128:(dc + 1) * 128, :])

        # ---- P2: attention --------------------------------------------
        # aT tiles (written per head-pair) share the qa tag with Q tiles.
        a_sb = []
        if do_p2:
            inv_sqrt_hd = 1.0 / float(np.sqrt(HD))
            for pair in range(NF):
                at = U.tile([128, SQ], BF16, name="qa", tag="qa",
                            bufs=qa_bufs)
                a_sb.append(at)
                for qsl in range(2):
                    qbase = qsl * 512
                    att = [P["psA"].tile([HE, 512], F32, name="psA",
                                         tag="psA") for _ in range(2)]
                    tl = list(range(8)) if qsl == 0 else list(range(16))
                    for t in tl:
                        # leading chunk covered by this k-tile (t//4 is the
                        # diagonal chunk; clamp to this q-slot's chunks)
                        c0g = max(2 * qsl, t // 4)
                        off = (c0g - 2 * qsl) * QC
                        d = t - 4 * c0g
                        ps = P["psS"].tile([128, 1024], F32, name="psS",
                                           tag="psS")
                        masked = 0 <= d <= 3
                        if masked:
                            # seed both banks with the additive mask on the
                            # diagonal sub-block (one identity LDWEIGHTS for
                            # both); the score matmuls accumulate there
                            # (has_written) and plain-write the rest.
                            for h in range(2):
                                nc.tensor.matmul(
                                    ps[:, h * 512 + off:h * 512 + off + QC],
                                    id_sb, mask_sb[d],
                                    start=True, stop=False,
                                    skip_group_check=True)
                        for h in range(2):
                            pb = h * 64
                            nc.tensor.matmul(
                                ps[:, h * 512 + off:(h + 1) * 512],
                                kt_sb[pair][pb:pb + 64,
                                            t * KT:(t + 1) * KT],
                                q_sb[pair][pb:pb + 64,
                                           qbase + off:qbase + 512],
                                start=not masked, stop=True,
                                skip_group_check=True)
                        num = U.tile([128, 1024], BF16, name="num",
                                     tag="num", bufs=4)
                        if off == 0:
                            nc.scalar.activation(num[:], ps[:], ACTF.Exp,
                                                 scale=inv_sqrt_hd)
                        else:
                            # both heads' tails in one strided instruction:
                            # cols [off:512] and [512+off:1024]
                            pv = ps.rearrange(
                                "p (a b) -> p a b", a=2)[:, :, off:512]
                            nv = num.rearrange(
                                "p (a b) -> p a b", a=2)[:, :, off:512]
                            nc.scalar.activation(nv, pv, ACTF.Exp,
                                                 scale=inv_sqrt_hd)
                        for h in range(2):
                            nc.tensor.matmul(
                                att[h][:, off:512],
                                v_sb[t][:, 2 * pair + h, :],
                                num[:, h * 512 + off:(h + 1) * 512],
                                start=(t == 0), stop=(t == tl[-1]))
                    # epilogue: normalize by denominator, write aT.  The
                    # odd head's result must land on partitions 64-127 of
                    # the aT tile; DVE lanes cannot shift partitions, so
                    # it goes through a small SBUF->SBUF DMA.
                    for h in range(2):
                        den = U.tile([1, 512], BF16, name="den", tag="den",
                                     bufs=2)
                        denf = U.tile([1, 512], F32, name="denf",
                                      tag="denf", bufs=2)
                        nc.vector.tensor_scalar_add(denf[:],
                                                    att[h][HD:HE, :],
                                                    SM_EPS)
                        nc.vector.reciprocal(den[:], denf[:])
                        # broadcast 1/den across partitions on the (idle)
                        # GpSimd engine, keeping PE/DVE off this chain
                        repS = U.tile([64, 512], BF16, name="repS",
                                      tag="repS", bufs=2)
                        nc.gpsimd.partition_broadcast(repS[:], den[0:1, :])
                        if h == 0:
                            nc.vector.tensor_mul(
                                at[0:64, qbase:qbase + 512],
                                att[0][0:HD, :], repS[:])
                        else:
                            tmp = U.tile([64, 512], BF16, name="atmp",
                                         tag="atmp", bufs=2)
                            nc.vector.tensor_mul(tmp[:], att[1][0:HD, :],
                                                 repS[:])
                            nc.sync.dma_start(
                                at[64:128, qbase:qbase + 512], tmp[:])

        # ---- P3: out-projection + bias + residual + LN stats ----------
        if do_p3:
            if not do_p2:
                a_sb = q_sb  # placeholder so p1+p34 builds run
            sum_ps = [P["psA"].tile([1, 512], F32, name="psA", tag="psA")
                      for _ in range(2)]
            ssq_ps = [P["psA"].tile([1, 512], F32, name="psA", tag="psA")
                      for _ in range(2)]
            for f in range(NF):
                ps = P["psS"].tile([128, 1024], F32, name="psS", tag="psS")
                for c in range(2):
                    for dc in range(NDC):
                        nc.tensor.matmul(
                            ps[:, c * 512:(c + 1) * 512],
                            wo_sb[dc][:, f * 128:(f + 1) * 128],
                            a_sb[dc][:, c * 512:(c + 1) * 512],
                            start=(dc == 0), stop=(dc == NDC - 1))
                zt = z_sb[f]
                nc.vector.scalar_tensor_tensor(
                    zt[:], in0=ps[:], scalar=bo_sb[f], in1=xq_sb[f][:],
                    op0=ALU.add, op1=ALU.add)
                sq = U.tile([128, SQ], BF16, name="sq", tag="sq", bufs=1)
                nc.scalar.activation(sq[:], zt[:], ACTF.Square)
                for c in range(2):
                    nc.tensor.matmul(sum_ps[c][0:1, :], ones_sb,
                                     zt[:, c * 512:(c + 1) * 512],
                                     start=(f == 0), stop=(f == NF - 1))
                    nc.tensor.matmul(ssq_ps[c][0:1, :], ones_sb,
                                     sq[:, c * 512:(c + 1) * 512],
                                     start=(f == 0), stop=(f == NF - 1))

        # ---- P4: LayerNorm normalize ----------------------------------
        if do_p4:
            lmean = U.tile([1, SQ], F32, name="lmean", tag="lnsc", bufs=3)
            msq = U.tile([1, SQ], F32, name="msq", tag="lnsc", bufs=3)
            rstd = U.tile([1, SQ], F32, name="rstd", tag="lnsc", bufs=3)
            for c in range(2):
                cs = c * 512
                nc.vector.tensor_scalar_mul(lmean[:, cs:cs + 512],
                                            sum_ps[c][0:1, :], 1.0 / D)
                nc.vector.tensor_scalar_mul(msq[:, cs:cs + 512],
                                            ssq_ps[c][0:1, :], 1.0 / D)
            nc.vector.tensor_mul(rstd[:], lmean[:], lmean[:])   # m^2
            nc.vector.tensor_sub(msq[:], msq[:], rstd[:])       # var
            nc.scalar.activation(rstd[:], msq[:], ACTF.Sqrt, bias=eps_sb[:])
            nc.vector.reciprocal(rstd[:], rstd[:])
            lmean_b = U.tile([1, SQ], BF16, name="lmean_b", tag="den",
                             bufs=2)
            rstd_b = U.tile([1, SQ], BF16, name="rstd_b", tag="den",
                            bufs=2)
            nc.vector.tensor_copy(lmean_b[:], lmean[:])
            nc.vector.tensor_copy(rstd_b[:], rstd[:])
            mrep = U.tile([128, SQ], BF16, name="mrep", tag="lnrep", bufs=2)
            rrep = U.tile([128, SQ], BF16, name="rrep", tag="lnrep", bufs=2)
            if p4_level >= 2:
                nc.gpsimd.partition_broadcast(mrep[:], lmean_b[0:1, :])
                nc.gpsimd.partition_broadcast(rrep[:], rstd_b[0:1, :])
            if p4_level >= 3:
                for f in range(NF):
                    t1 = U.tile([128, SQ], F32, name="t1", tag="t1", bufs=1)
                    t1b = U.tile([128, SQ], BF16, name="t1b", tag="t1b",
                                 bufs=2)
                    nc.gpsimd.tensor_sub(t1[:], z_sb[f][:], mrep[:])
                    nc.vector.tensor_mul(t1[:], t1[:], rrep[:])
                    nc.vector.tensor_scalar(t1b[:], t1[:], ga_sb[f],
                                            be_sb[f], ALU.mult, ALU.add)
                    if p4_level >= 4:
                        nc.sync.dma_start(yT[f * 128:(f + 1) * 128, :],
                                          t1b[:])
        ctx_lp.__exit__(None, None, None)

    from contextlib import ExitStack
    with tile.TileContext(nc) as tc:
        with ExitStack() as ctx:
            P = {
                "u": ctx.enter_context(tc.tile_pool(name="u", bufs=2)),
                "psS": ctx.enter_context(
                    tc.tile_pool(name="psS", bufs=2, space="PSUM")),
                "psA": ctx.enter_context(
                    tc.tile_pool(name="psA", bufs=4, space="PSUM")),
            }
            if n_iters > 1:
                with tc.For_i(0, n_iters, 1,
                              hint_engines=(mybir.EngineType.PE,
                                            mybir.EngineType.DVE,
                                            mybir.EngineType.Activation,
                                            mybir.EngineType.SP)):
                    emit(tc, P)
            else:
                emit(tc, P)
    nc.compile()
    return nc


# ----------------------------------------------------------------------------
# Host wrapper
# ----------------------------------------------------------------------------

_CACHE = {}


def _get_program(n_iters=1, timing_mode=False, phases=("p1", "p2", "p34")):
    key = (n_iters, timing_mode, tuple(phases))
    if key not in _CACHE:
        _CACHE[key] = _build_program(n_iters=n_iters, timing_mode=timing_mode,
                                     phases=phases)
    return _CACHE[key]


def _prep_inputs(x, attn_mask, W_qkv, b_qkv, W_out, b_out, gamma, beta):
    f32 = np.float32
    x = np.asarray(x, f32)
    attn_mask = np.asarray(attn_mask, f32)
    wq = np.ascontiguousarray(W_qkv[:, 0:D]).astype(BF)
    wk = np.ascontiguousarray(W_qkv[:, D:2 * D]).astype(BF)
    wv = np.ascontiguousarray(W_qkv[:, 2 * D:3 * D]).astype(BF)
    wo = np.ascontiguousarray(W_out).astype(BF)
    b_qkv = np.asarray(b_qkv, f32)
    cstf = np.zeros((128, 5 * NF), f32)
    for f in range(NF):
        cstf[:, 0 * NF + f] = b_qkv[f * 128:(f + 1) * 128]
        cstf[:, 1 * NF + f] = b_qkv[D + f * 128:D + (f + 1) * 128]
        cstf[:, 2 * NF + f] = np.asarray(b_out, f32)[f * 128:(f + 1) * 128]
        cstf[:, 3 * NF + f] = np.asarray(gamma, f32)[f * 128:(f + 1) * 128]
        cstf[:, 4 * NF + f] = np.asarray(beta, f32)[f * 128:(f + 1) * 128]
    inv_sqrt_hd = 1.0 / np.sqrt(HD)

    in_maps = []
    for core in range(N_CORES):
        bb, sh = core // 2, core % 2
        qsel = np.arange(sh, S, 2)
        xb = x[bb]
        xqT = np.ascontiguousarray(xb[qsel].T)
        # mask tiles: block d of the diagonal band, taken from chunk 0
        # (queries 2*j+sh, keys 128*d+kl); identical for all chunks by
        # the stride-2 causal structure (verified in kernel()).
        cstb = np.zeros((128, 2304), BF)
        cstb[:, 0:D] = np.broadcast_to(b_qkv[2 * D:3 * D][None, :],
                                       (128, D)).astype(BF)
        for dd in range(4):
            blk = attn_mask[bb, qsel[0:QC], dd * KT:(dd + 1) * KT]
            cstb[:, D + dd * QC:D + (dd + 1) * QC] = blk.T.astype(BF)
        cstb[:, 2048:2176] = np.ones((128, 128), BF)
        cstb[:, 2176:2304] = np.eye(128, dtype=BF)
        in_maps.append({
            "xT_b": np.ascontiguousarray(xb.T).astype(BF),
            "xqT_b": xqT.astype(BF),
            "wq_b": wq, "wk_b": wk, "wv_b": wv, "wo_b": wo,
            "cstf": cstf, "cstb": cstb,
        })
    return in_maps


def _check_mask(attn_mask):
    """Verify the stride-2-uniform causal-band structure we compile for."""
    m = np.asarray(attn_mask, np.float32)
    for bb in (0, B - 1):
        for sh in (0, 1):
            qsel = np.arange(sh, S, 2)
            ref = m[bb, qsel[0:QC], 0:4 * KT]          # chunk 0 band
            for c in (1, NCH - 1):
                qs = qsel[c * QC:(c + 1) * QC]
                band = m[bb, qs, 4 * c * KT:4 * (c + 1) * KT]
                if not np.array_equal(band, ref):
                    return False
            # below band: zeros; above band: very negative
            if m[bb, qsel[QC:2 * QC], 0:4 * KT].min() != 0.0:
                return False
            if m[bb, qsel[0:QC], 4 * KT:].max() > -1e8:
                return False
    return True


def kernel(x, attn_mask, W_qkv, b_qkv, W_out, b_out, gamma, beta,
           n_iters=1):
    assert _check_mask(attn_mask), "mask is not stride-2 causal-uniform"
    nc = _get_program(n_iters=n_iters)
    in_maps = _prep_inputs(x, attn_mask, W_qkv, b_qkv, W_out, b_out,
                           gamma, beta)
    res = run_bass_kernel_spmd(nc, in_maps, list(range(N_CORES)))
    out = np.empty((B, S, D), np.float32)
    for core in range(N_CORES):
        bb, sh = core // 2, core % 2
        out[bb, sh::2] = res.results[core]["yT"].T.astype(np.float32)
    return out
